# revision 1
# baseline (speedup 1.0000x reference)
"""Trainium2 Bass kernel for nn_ButterflyLayer2D (butterfly 2D CNN).

Strategy: pure data parallel over 8 NeuronCores (16 batch each), with the
per-core batch processed in 2 phases of 8 to fit SBUF.

All tensors are pre-arranged on the host (numpy) into DMA-friendly layouts:
  - activations live in SBUF as [128 = (w%2)*64 + c, (node, b, h, w//2)]
    so each 2x2-stride-2 per-node conv becomes 4 fp32r matmuls with K=128
    chunks: col-group q = output w-parity (tile_position (0, 64q)), x-chunks
    accumulate in PSUM. PSUM [128=(q,c_out), N] is evicted full-width by a
    single relu+bias op (alternating ScalarE/VectorE) directly into the next
    level's interleaved layout — zero data reshuffling anywhere on chip.
  - the input 4x4-patch conv uses the same trick with K=16 row-groups spread
    over 4 partition groups (one per b%4) for PE concurrency.
  - the final per-node dense is a [64,128] x [64,b] matmul; outputs are
    written as [128=(r,ou,ov), (ph,node,b)] and decoded on the host.
Weights are streamed from HBM in 8-node chunks through a recycled tile tag.
"""

import numpy as np
from contextlib import ExitStack

import concourse.bass as bass
import concourse.tile as tile
from concourse import bacc, mybir
from concourse.bass_utils import run_bass_kernel_spmd

F32 = mybir.dt.float32
F32R = mybir.dt.float32r
BF16 = mybir.dt.bfloat16
AF = mybir.ActivationFunctionType
ALU = mybir.AluOpType

B, IN, NLVL, KLVL, C = 128, 256, 6, 3, 64
TCOL = 1024               # psum tile columns
PBUFS = 4                 # psum tile bufs
NK, OU, OV = 8, 8, 8
NCORES = 8
BC = B // NCORES          # 16 per-core batch
PH = 1                    # phases per core
BG = BC // PH             # batch per phase
HALF = BG // 4            # input-conv b-subgroups per partition group
LVL_NODES = [4, 16, 64, 64, 64, 64]          # nodes per level
LVL_HIN = [64, 32, 16, 8, 4, 2]              # spatial H into each level
WGRP = 8                  # weight streaming chunk (nodes)


# ----------------------------------------------------------------------------
# host-side pre-arrangement
# ----------------------------------------------------------------------------

def _prep_weights(inputs):
    """Weights/biases blobs shared by all cores."""
    out = {}
    # input filter: lhsT [16=(p,q), 64], replicated at partition bases 0/32/64/96
    import ml_dtypes
    fin = inputs["in_filter"][:, :, 0, :].reshape(16, C).astype(np.float32)
    finr = np.zeros((128, C), np.float32)
    for g in range(4):
        finr[g * 32 : g * 32 + 16] = fin
    out["fin"] = finr.astype(ml_dtypes.bfloat16)
    out["bin"] = np.concatenate([inputs["in_bias"], inputs["in_bias"]]).reshape(
        128, 1
    ).astype(np.float32)

    for lvl in range(1, NLVL + 1):
        f = inputs[f"f{lvl}"].astype(np.float32)  # [n,n,2,2,C,C] (x,y,ci,co)
        n = f.shape[0]
        assert n == 2 ** min(lvl, KLVL)
        # lhsT per node: [(y*64+ci), (x*64+co)]
        w = f.transpose(0, 1, 3, 4, 2, 5).reshape(n * n, 2 * C, 2 * C)
        if lvl <= KLVL:
            # sibling-pair blob: per pair (u,2t)+(u,2t+1):
            # [(y,ci)=128, (x, coA|coB)=256] -> [128, pairs*256]
            wp = w.reshape(n * n // 2, 2, 2 * C, 2, C)  # [pair, s, (y,ci), x, co]
            wp = wp.transpose(2, 0, 3, 1, 4)            # [(y,ci), pair, x, s, co]
            out[f"w{lvl}"] = np.ascontiguousarray(wp).reshape(
                128, n * n * 128
            ).astype(ml_dtypes.bfloat16)
        else:
            # blob [128, nodes*128], free = (node, x*64+co)
            out[f"w{lvl}"] = np.ascontiguousarray(w.transpose(1, 0, 2)).reshape(
                128, n * n * 128
            ).astype(ml_dtypes.bfloat16)
        b = inputs[f"b{lvl}"].astype(np.float32).reshape(n * n, C)
        if lvl < NLVL:
            # [128, nodes]: rows (q,c) with bias duplicated across q
            bb = np.concatenate([b, b], axis=1)  # [nodes, 128]
            out[f"b{lvl}"] = np.ascontiguousarray(bb.T)
        else:
            # lvl6 node-pair scheme: psum rows = (cA, cB) for pair (2k, 2k+1)
            bb = b.reshape(n * n // 2, 2 * C)  # [pairs, (cA,cB)]
            out[f"b{lvl}"] = np.ascontiguousarray(bb.T)  # [128, 32]
    # dense: lhsT per node [64=c, 128=(r, ou*8+ov)]
    wd = inputs["Wd"].astype(np.float32).reshape(NK * NK, 2, C, OU * OV)
    wd = wd.transpose(2, 0, 1, 3).reshape(C, NK * NK * 2 * OU * OV)
    out["wd"] = np.ascontiguousarray(wd).astype(ml_dtypes.bfloat16)
    return out


def _prep_input(in_data_core):
    """Per-core input blob: [64 = (b%4)*16 + (i%4)*4 + (j%4),
    (ph, b//4%2, x=i//4, y4=j//4)] packed (no zero rows)."""
    ind = in_data_core[:, :, :, 0]  # [16, 256, 256]
    a = ind.reshape(PH, HALF, 4, 64, 4, 64, 4)  # [ph, half, g, x, p, y4, q]
    a = a.transpose(2, 4, 6, 0, 1, 3, 5)        # [g, p, q, ph, half, x, y4]
    import ml_dtypes
    return np.ascontiguousarray(a).reshape(64, PH * HALF * 64 * 64).astype(ml_dtypes.bfloat16)


def _decode_output(t2_core):
    """t2 [128=(r,ou,ov), (ph, node, bl)] -> [16, 64, 64, 2]."""
    t = t2_core.reshape(2, OU, OV, PH, NK, NK, BG)  # r,ou,ov,ph,u,v,bl
    t = t.transpose(3, 6, 4, 1, 5, 2, 0)            # ph,bl,u,ou,v,ov,r
    return np.ascontiguousarray(t).reshape(BC, NK * OU, NK * OV, 2)


# ----------------------------------------------------------------------------
# device kernel
# ----------------------------------------------------------------------------

def _build_kernel(reps=1, xouter=True):
    nc = bacc.Bacc(None, target_bir_lowering=False)
    p = {}
    p["a0"] = nc.declare_dram_parameter("a0", [64, PH * HALF * 64 * 64], BF16, isOutput=False)
    p["fin"] = nc.declare_dram_parameter("fin", [128, C], BF16, isOutput=False)
    p["bin"] = nc.declare_dram_parameter("bin", [128, 1], F32, isOutput=False)
    for lvl in range(1, NLVL + 1):
        n2 = LVL_NODES[lvl - 1]
        p[f"w{lvl}"] = nc.declare_dram_parameter(f"w{lvl}", [128, n2 * 128], BF16, isOutput=False)
        bcols = n2 if lvl < NLVL else n2 // 2
        p[f"b{lvl}"] = nc.declare_dram_parameter(f"b{lvl}", [128, bcols], F32, isOutput=False)
    p["wd"] = nc.declare_dram_parameter("wd", [64, NK * NK * 128], BF16, isOutput=False)
    t2 = nc.declare_dram_parameter("t2", [128, PH * NK * NK * BG], F32, isOutput=True)

    evict_ctr = [0]

    def evict(out_ap, psum_ap, bias_ap):
        """relu(psum + bias) -> sbuf, alternating engines to split the load."""
        evict_ctr[0] += 1
        if evict_ctr[0] % 2 == 0:
            nc.scalar.activation(out_ap, psum_ap, AF.Relu, bias=bias_ap)
        else:
            nc.vector.tensor_scalar(out_ap, psum_ap, bias_ap, 0.0,
                                    op0=ALU.add, op1=ALU.max)

    with tile.TileContext(nc) as tc, ExitStack() as ctx:
        const = ctx.enter_context(tc.tile_pool(name="const", bufs=1))
        wpool = ctx.enter_context(tc.tile_pool(name="wts", bufs=4))
        apool = ctx.enter_context(tc.tile_pool(name="acts", bufs=1))
        inpool = ctx.enter_context(tc.tile_pool(name="inp", bufs=1))
        fpool = ctx.enter_context(tc.tile_pool(name="feat", bufs=2))
        ppool = ctx.enter_context(tc.tile_pool(name="ps", bufs=PBUFS, space="PSUM"))
        spool = ppool

        # constants: input filter, biases (all small, loaded once)
        fin_t = const.tile([128, C], BF16)
        nc.sync.dma_start(fin_t[:], p["fin"][:])
        bin_t = const.tile([128, 1], F32)
        nc.sync.dma_start(bin_t[:], p["bin"][:])
        bias_t = {}
        for lvl in range(1, NLVL + 1):
            bcols = LVL_NODES[lvl - 1] if lvl < NLVL else LVL_NODES[lvl - 1] // 2
            bias_t[lvl] = const.tile([128, bcols], F32, tag=f"bias{lvl}", name=f"bias{lvl}")
            nc.sync.dma_start(bias_t[lvl][:], p[f"b{lvl}"][:])

        for phx in range(reps * PH):
            ph = phx % PH
            # ---------------- input staging ----------------
            a0s = inpool.tile([128, HALF * 64 * 64], BF16, tag="a0s", name=f"a0s{phx}")
            for g in range(4):
                nc.sync.dma_start(
                    a0s[g * 32 : g * 32 + 16, :],
                    p["a0"][g * 16 : (g + 1) * 16,
                            ph * HALF * 64 * 64 : (ph + 1) * HALF * 64 * 64],
                )
            a0v = a0s[:].rearrange("p (h x y) -> p h x y", h=HALF, x=64)

            # ---------------- input conv ----------------
            # X slab: [128=(y%2,c), (bl, x, y2)]  (bl=8, x=64, y2=32)
            X = apool.tile([128, BG * 64 * 32], BF16, tag="s0", name=f"x{phx}")
            Xv = X[:].rearrange("p (b h w) -> p b h w", b=BG, h=64)
            for bl in range(BG):
                g, half = bl % 4, bl // 4
                for xh in range(2048 // TCOL):
                    pt = ppool.tile([128, TCOL], F32, tag="ps",
                                    padded_shape=[128, TCOL],
                                    name=f"pin{phx}_{bl}_{xh}")
                    for sub in range(TCOL // 512):
                        xq = xh * (TCOL // 512) + sub
                        for q in (0, 1):
                            rhs = a0v[g * 32 : g * 32 + 16, half,
                                      xq * 16 : (xq + 1) * 16, q::2]
                            nc.tensor.matmul(
                                pt[q * 64 : (q + 1) * 64,
                                   sub * 512 : (sub + 1) * 512],
                                fin_t[g * 32 : g * 32 + 16, :],
                                rhs,
                                start=True, stop=True,
                                tile_position=(g * 32, q * 64),
                            )
                    evict(Xv[:, bl, xh * (TCOL // 32) : (xh + 1) * (TCOL // 32), :], pt[:], bin_t[:, 0:1])

            # ---------------- levels 1..5 (q-scheme) ----------------
            cur = X          # slab with free = (node, bl, h, w2)
            cur_nodes = 1
            tags = ["s1", "s0", "s1", "s0", "s1"]
            for lvl in range(1, 6):
                n2 = LVL_NODES[lvl - 1]
                grid = int(np.sqrt(n2))
                Hin = LVL_HIN[lvl - 1]
                W2in = Hin // 2
                Ho, W2o = Hin // 2, W2in // 2
                ncols_out = BG * Ho * W2o
                nxt = apool.tile([128, n2 * ncols_out], BF16,
                                 tag=tags[lvl - 1], name=f"a{lvl}_{phx}")
                curv = cur[:].rearrange("p (n b h w) -> p n b h w",
                                        n=cur_nodes, b=BG, h=Hin)
                nxtv = nxt[:].rearrange("p (n b h w) -> p n b h w",
                                        n=n2, b=BG, h=Ho)
                pgrid = int(np.sqrt(cur_nodes))
                if lvl <= KLVL:
                    # sibling-pair scheme: M=128=(coA,coB), shared parent rhs
                    Wo = W2in          # output width = rhs w-count
                    npairs = n2 // 2
                    # block = (bper b, hper h, all Wo) == 1024 cols (2 banks)
                    hper = min(Ho, TCOL // Wo)
                    bper = min(BG, max(1, TCOL // (Wo * hper)))
                    ncol = bper * hper * Wo
                    PGRP = 4           # pairs per weight DMA chunk
                    for g0 in range(0, npairs, PGRP):
                        gn = min(PGRP, npairs - g0)
                        wlt = wpool.tile([128, PGRP * 256], BF16, tag="wch",
                                         name=f"w{lvl}_{phx}_{g0}")
                        nc.sync.dma_start(
                            wlt[:, : gn * 256],
                            p[f"w{lvl}"][:, g0 * 256 : (g0 + gn) * 256],
                        )
                        for pr in range(g0, g0 + gn):
                            u, t = pr // (grid // 2), pr % (grid // 2)
                            nA = u * grid + 2 * t
                            nB = nA + 1
                            lp_ = pr - g0
                            pnode = (u // 2) * pgrid + t
                            # sub-splitting along b (or h) into 512-col chunks
                            nsub = ncol // 512
                            if bper >= nsub:
                                sb, sh = bper // nsub, hper
                            else:
                                sb, sh = 1, hper // (nsub // max(1, bper))
                            hsubs = hper // sh
                            for bs in range(0, BG, bper):
                                for h0 in range(0, Ho, hper):
                                    pt = ppool.tile(
                                        [128, ncol], F32, tag="ps",
                                        padded_shape=[128, TCOL],
                                        name=f"p{lvl}_{phx}_{pr}_{bs}_{h0}")
                                    for sub in range(nsub):
                                        b1 = bs + (sub // hsubs) * sb
                                        h1 = h0 + (sub % hsubs) * sh
                                        for x in (0, 1):
                                            rhs = curv[:, pnode, b1 : b1 + sb,
                                                       2 * h1 + x : 2 * (h1 + sh) : 2,
                                                       :]
                                            nc.tensor.matmul(
                                                pt[:, sub * 512 : (sub + 1) * 512],
                                                wlt[:, lp_ * 256 + x * 128 :
                                                    lp_ * 256 + (x + 1) * 128],
                                                rhs,
                                                start=(x == 0), stop=(x == 1),
                                            )
                                    for shalf, node in ((0, nA), (1, nB)):
                                        ptv = pt[shalf * 64 : (shalf + 1) * 64, :] \
                                            .rearrange("c (b h w) -> c b h w",
                                                       b=bper, h=hper)
                                        bias_ap = bias_t[lvl][
                                            shalf * 64 : (shalf + 1) * 64,
                                            node : node + 1]
                                        for par in (0, 1):
                                            evict(
                                                nxtv[par * 64 : (par + 1) * 64,
                                                     node, bs : bs + bper,
                                                     h0 : h0 + hper, :],
                                                ptv[:, :, :, par::2],
                                                bias_ap,
                                            )
                else:
                    # q-scheme (deep levels)
                    nblk = max(1, ncols_out // 512)
                    bper = BG // nblk
                    ncol = bper * Ho * W2o
                    for g0 in range(0, n2, WGRP):
                        gn = min(WGRP, n2 - g0)
                        wlt = wpool.tile([128, WGRP * 128], BF16, tag="wch",
                                         name=f"w{lvl}_{phx}_{g0}")
                        nc.sync.dma_start(
                            wlt[:, : gn * 128],
                            p[f"w{lvl}"][:, g0 * 128 : (g0 + gn) * 128],
                        )
                        for node in range(g0, g0 + gn):
                            ln = node - g0
                            pnode = node
                            for blk in range(nblk):
                                bs = blk * bper
                                pt = ppool.tile([128, ncol], F32, tag="ps",
                                                padded_shape=[128, TCOL],
                                                name=f"p{lvl}_{phx}_{node}_{blk}")
                                qx = [(x, q) for x in (0, 1) for q in (0, 1)] \
                                    if xouter else \
                                    [(x, q) for q in (0, 1) for x in (0, 1)]
                                for x, q in qx:
                                    rhs = curv[:, pnode, bs : bs + bper, x::2, q::2]
                                    nc.tensor.matmul(
                                        pt[q * 64 : (q + 1) * 64, :],
                                        wlt[:, ln * 128 + x * 64 :
                                            ln * 128 + (x + 1) * 64],
                                        rhs,
                                        start=(x == 0), stop=(x == 1),
                                        skip_group_check=xouter,
                                        tile_position=(0, q * 64),
                                    )
                                evict(
                                    nxtv[:, node, bs : bs + bper, :, :],
                                    pt[:],
                                    bias_t[lvl][:, node : node + 1],
                                )
                cur = nxt
                cur_nodes = n2

            # ---------------- level 6 (node pairs, M=64) ----------------
            # cur: [128, (n=64, bl, h=2, w2=1)] ; feats F [64=c, (node, bl)]
            F = fpool.tile([64, NK * NK * BG], BF16, tag="feats", name=f"f{phx}")
            Fv = F[:].rearrange("c (n b) -> c n b", n=NK * NK)
            curv = cur[:].rearrange("p (n b h w) -> p n b h w", n=64, b=BG, h=2)
            for g0 in range(0, 64, WGRP):
                w6t = wpool.tile([128, WGRP * 128], BF16, tag="wch",
                                 name=f"w6_{phx}_{g0}")
                nc.sync.dma_start(
                    w6t[:], p["w6"][:, g0 * 128 : (g0 + WGRP) * 128]
                )
                for pr in range(g0 // 2, (g0 + WGRP) // 2):
                    nA, nB = 2 * pr, 2 * pr + 1
                    pt = spool.tile([128, BG], F32, tag="ps", padded_shape=[128, TCOL],
                                    name=f"p6_{phx}_{pr}")
                    hx = [(h_, x_) for x_ in (0, 1) for h_ in (0, 1)] \
                        if xouter else \
                        [(h_, x_) for h_ in (0, 1) for x_ in (0, 1)]
                    for half, x in hx:
                        node = nA if half == 0 else nB
                        ln = node - g0
                        rhs = curv[:, node, :, x, 0]
                        nc.tensor.matmul(
                            pt[half * 64 : (half + 1) * 64, :],
                            w6t[:, ln * 128 + x * 64 :
                                ln * 128 + (x + 1) * 64],
                            rhs,
                            start=(x == 0), stop=(x == 1),
                            skip_group_check=xouter,
                            tile_position=(0, half * 64),
                        )
                    bias_ap = bias_t[6][:, pr : pr + 1]
                    evict_ctr[0] += 1
                    if evict_ctr[0] % 2 == 0:
                        nc.scalar.activation(Fv[0:64, nA, :], pt[0:64, :], AF.Relu,
                                             bias=bias_ap[0:64, :])
                        nc.scalar.activation(Fv[0:64, nB, :], pt[64:128, :], AF.Relu,
                                             bias=bias_ap[64:128, :])
                    else:
                        nc.vector.tensor_scalar(Fv[0:64, nA, :], pt[0:64, :],
                                                bias_ap[0:64, :], 0.0,
                                                op0=ALU.add, op1=ALU.max)
                        nc.vector.tensor_scalar(Fv[0:64, nB, :], pt[64:128, :],
                                                bias_ap[64:128, :], 0.0,
                                                op0=ALU.add, op1=ALU.max)

            # ---------------- dense ----------------
            t2s = fpool.tile([128, NK * NK * BG], F32, tag="t2s", name=f"t2s{phx}")
            t2sv = t2s[:].rearrange("m (n b) -> m n b", n=NK * NK)
            for g0 in range(0, 64, WGRP):
                wdt = wpool.tile([64, WGRP * 128], BF16, tag="wdch",
                                 name=f"wd_{phx}_{g0}")
                nc.sync.dma_start(
                    wdt[:], p["wd"][:, g0 * 128 : (g0 + WGRP) * 128]
                )
                for node in range(g0, g0 + WGRP):
                    ln = node - g0
                    pt = spool.tile([128, BG], F32, tag="ps", padded_shape=[128, TCOL],
                                    name=f"pd_{phx}_{node}")
                    nc.tensor.matmul(
                        pt[:],
                        wdt[:, ln * 128 : (ln + 1) * 128],
                        Fv[:, node, :],
                        start=True, stop=True,
                    )
                    evict_ctr[0] += 1
                    if evict_ctr[0] % 2 == 0:
                        nc.scalar.copy(t2sv[:, node, :], pt[:])
                    else:
                        nc.vector.tensor_copy(t2sv[:, node, :], pt[:])
            nc.sync.dma_start(
                t2[:, ph * NK * NK * BG : (ph + 1) * NK * NK * BG], t2s[:]
            )
    nc.compile()
    return nc


# ----------------------------------------------------------------------------
# entry point
# ----------------------------------------------------------------------------

def kernel(**inputs):
    inputs = {k: np.asarray(v) for k, v in inputs.items()}
    wblobs = _prep_weights(inputs)
    nc = _build_kernel()
    in_maps = []
    for c in range(NCORES):
        m = dict(wblobs)
        m["a0"] = _prep_input(inputs["in_data"][c * BC : (c + 1) * BC])
        in_maps.append(m)
    res = run_bass_kernel_spmd(nc, in_maps, list(range(NCORES)))
    outs = [_decode_output(res.results[c]["t2"]) for c in range(NCORES)]
    return np.concatenate(outs, axis=0).astype(np.float32)


if __name__ == "__main__":
    import reference as ref

    inputs = {k: np.asarray(v) for k, v in ref.setup_inputs().items()}
    expected = np.asarray(ref.reference(**inputs))
    actual = kernel(**inputs)
    err = np.abs(actual - expected).max()
    rel = err / np.abs(expected).max()
    print("absmax:", err, "rel:", rel)



# revision 6
# speedup vs baseline: 1.3315x; 1.3315x over previous
"""Trainium2 Bass kernel for nn_ButterflyLayer2D (butterfly 2D CNN).

Strategy: pure data parallel over 8 NeuronCores (16 batch each).

Layouts (per core):
  - activations in SBUF as [128 = (w%2)*64 + c, (node, b, h, w//2)] so each
    2x2-stride-2 per-node conv is a q-parity scheme: K=128=(y,ci) in
    partitions, x (h-parity) accumulated over 2 matmuls, q (output w-parity)
    split over two col-tiled M=64 matmuls running concurrently on the PE.
    PSUM partitions then equal the destination layout, so every PSUM tile is
    evicted by ONE full-width contiguous relu+bias op (alternating
    ScalarE/VectorE) — this keeps the eviction engines at ~half the load of
    the sibling-pair scheme.
  - input 4x4-patch conv: patches paired along y so K=32 (2 patches x 16)
    with block-diagonal weights and M=128=(y%2, c); 4 batch-groups run
    concurrently at row offsets 0/32/64/96. Input DMA blob is a fully-packed
    [128, 8192] slab.
  - level 6 produces feats F2 [128=(node%2, c), (pair, b)]; the dense layer
    is row-tiled: even/odd nodes of a pair at row offsets 0/64 run
    concurrently with per-node [64,128] weights.
  - deep weights (w4, w6, wd) are prefetched at kernel start; w5 reuses the
    input-blob SBUF slot once the input conv is done.
"""

import numpy as np
from contextlib import ExitStack

import concourse.bass as bass
import concourse.tile as tile
from concourse import bacc, mybir
from concourse.bass_utils import run_bass_kernel_spmd

F32 = mybir.dt.float32
BF16 = mybir.dt.bfloat16
AF = mybir.ActivationFunctionType
ALU = mybir.AluOpType

B, IN, NLVL, KLVL, C = 128, 256, 6, 3, 64
NK, OU, OV = 8, 8, 8
NCORES = 8
BC = B // NCORES          # 16 per-core batch
BG = BC                   # batch per phase (single phase)
LVL_NODES = [4, 16, 64, 64, 64, 64]          # nodes per level
LVL_HIN = [64, 32, 16, 8, 4, 2]              # spatial H into each level
WGRP = 8                  # w3 streaming chunk (nodes)


# ----------------------------------------------------------------------------
# host-side pre-arrangement
# ----------------------------------------------------------------------------

def _prep_weights(inputs):
    """Weights/biases blobs shared by all cores."""
    import ml_dtypes
    out = {}
    # input filter: block-diagonal pair lhsT [32=(s,p,q), 128=(s,c)],
    # replicated at row bases 0/32/64/96 (4 concurrent batch groups)
    fin = inputs["in_filter"][:, :, 0, :].reshape(16, C).astype(np.float32)
    blk = np.zeros((32, 128), np.float32)
    blk[0:16, 0:64] = fin
    blk[16:32, 64:128] = fin
    finr = np.zeros((128, 128), np.float32)
    for g in range(4):
        finr[g * 32 : (g + 1) * 32] = blk
    out["fin"] = finr.astype(ml_dtypes.bfloat16)
    out["bin"] = np.concatenate([inputs["in_bias"], inputs["in_bias"]]).reshape(
        128, 1
    ).astype(np.float32)

    for lvl in range(1, NLVL + 1):
        f = inputs[f"f{lvl}"].astype(np.float32)  # [n,n,2,2,C,C] (x,y,ci,co)
        n = f.shape[0]
        assert n == 2 ** min(lvl, KLVL)
        # lhsT per node: [(y*64+ci), (x*64+co)] -> blob [128, n2*128]
        w = f.transpose(0, 1, 3, 4, 2, 5).reshape(n * n, 2 * C, 2 * C)
        out[f"w{lvl}"] = np.ascontiguousarray(w.transpose(1, 0, 2)).reshape(
            128, n * n * 128
        ).astype(ml_dtypes.bfloat16)
        b = inputs[f"b{lvl}"].astype(np.float32).reshape(n * n, C)
        if lvl < NLVL:
            # [128, nodes]: rows (q,c) with bias duplicated across q
            bb = np.concatenate([b, b], axis=1)  # [nodes, 128]
            out[f"b{lvl}"] = np.ascontiguousarray(bb.T)
        else:
            # lvl6 node-pair scheme: psum rows = (cA, cB) for pair (2k, 2k+1)
            bb = b.reshape(n * n // 2, 2 * C)  # [pairs, (cA,cB)]
            out[f"b{lvl}"] = np.ascontiguousarray(bb.T)  # [128, 32]
    # dense row-tiled: wd2 [128 = s*64 + c, pair*128 + (r*64 + ou*8 + ov)]
    wd = inputs["Wd"].astype(np.float32).reshape(NK * NK, 2, C, OU * OV)
    # [node, r, c, k2] -> [s, c, pair, (r,k2)]
    wd = wd.reshape(32, 2, 2, C, OU * OV)          # [pair, s, r, c, k2]
    wd = wd.transpose(1, 3, 0, 2, 4)               # [s, c, pair, r, k2]
    out["wd"] = np.ascontiguousarray(wd).reshape(128, 32 * 128).astype(
        ml_dtypes.bfloat16
    )
    return out


def _prep_input(in_data_core):
    """Per-core input blob [128, 8192]:
    row = (b%4)*32 + s*16 + p*4 + q ; col = (b//4)*2048 + x*32 + t
    value = in[b, 4x+p, 8t+4s+q]."""
    import ml_dtypes
    ind = in_data_core[:, :, :, 0]  # [16, 256, 256]
    a = ind.reshape(4, 4, 64, 4, 32, 2, 4)      # [half, g, x, p, t, s, q]
    a = a.transpose(1, 5, 3, 6, 0, 2, 4)        # [g, s, p, q, half, x, t]
    return np.ascontiguousarray(a).reshape(128, 8192).astype(ml_dtypes.bfloat16)


def _decode_output(t2_core):
    """t2 [128=(r,ou,ov), (s, pair, b)] -> [16, 64, 64, 2]."""
    t = t2_core.reshape(2, OU, OV, 2, 32, BG)   # r,ou,ov,s,p,b
    t = t.transpose(4, 3, 5, 0, 1, 2)           # p,s,b,r,ou,ov
    t = np.ascontiguousarray(t).reshape(8, 8, BG, 2, OU, OV)  # u,v,b,r,ou,ov
    t = t.transpose(2, 0, 4, 1, 5, 3)           # b,u,ou,v,ov,r
    return np.ascontiguousarray(t).reshape(BC, NK * OU, NK * OV, 2)


# ----------------------------------------------------------------------------
# device kernel
# ----------------------------------------------------------------------------

def _build_kernel(zero_deep_bias=True):
    nc = bacc.Bacc(None, target_bir_lowering=False)
    p = {}
    p["a0"] = nc.declare_dram_parameter("a0", [128, 8192], BF16, isOutput=False)
    p["fin"] = nc.declare_dram_parameter("fin", [128, 128], BF16, isOutput=False)
    p["bin"] = nc.declare_dram_parameter("bin", [128, 1], F32, isOutput=False)
    for lvl in range(1, NLVL + 1):
        n2 = LVL_NODES[lvl - 1]
        p[f"w{lvl}"] = nc.declare_dram_parameter(f"w{lvl}", [128, n2 * 128], BF16, isOutput=False)
        bcols = n2 if lvl < NLVL else n2 // 2
        p[f"b{lvl}"] = nc.declare_dram_parameter(f"b{lvl}", [128, bcols], F32, isOutput=False)
    p["wd"] = nc.declare_dram_parameter("wd", [128, 32 * 128], BF16, isOutput=False)
    t2 = nc.declare_dram_parameter("t2", [128, 2 * 32 * BG], F32, isOutput=True)

    evict_ctr = [0]

    def evict(out_ap, psum_ap, bias_ap):
        """relu(psum + bias) -> sbuf, alternating engines to split the load.
        bias_ap None -> plain relu."""
        evict_ctr[0] += 1
        if evict_ctr[0] % 2 == 0:
            if bias_ap is None:
                nc.scalar.activation(out_ap, psum_ap, AF.Relu)
            else:
                nc.scalar.activation(out_ap, psum_ap, AF.Relu, bias=bias_ap)
        else:
            if bias_ap is None:
                nc.vector.tensor_scalar_max(out_ap, psum_ap, 0.0)
            else:
                nc.vector.tensor_scalar(out_ap, psum_ap, bias_ap, 0.0,
                                        op0=ALU.add, op1=ALU.max)

    with tile.TileContext(nc) as tc, ExitStack() as ctx:
        const = ctx.enter_context(tc.tile_pool(name="const", bufs=1))
        wpool = ctx.enter_context(tc.tile_pool(name="wts", bufs=3))
        apool = ctx.enter_context(tc.tile_pool(name="acts", bufs=1))
        inpool = ctx.enter_context(tc.tile_pool(name="inp", bufs=1))
        pfpool = ctx.enter_context(tc.tile_pool(name="pf", bufs=2))
        pdpool = ctx.enter_context(tc.tile_pool(name="pfd", bufs=1))
        fpool = ctx.enter_context(tc.tile_pool(name="feat", bufs=1))
        opool = ctx.enter_context(tc.tile_pool(name="outp", bufs=2))
        psA = ctx.enter_context(tc.tile_pool(name="psA", bufs=4, space="PSUM"))
        psB = ctx.enter_context(tc.tile_pool(name="psB", bufs=2, space="PSUM"))

        # ---------------- constants ----------------
        fin_t = const.tile([128, 128], BF16)
        nc.sync.dma_start(fin_t[:], p["fin"][:])
        bin_t = const.tile([128, 1], F32)
        nc.sync.dma_start(bin_t[:], p["bin"][:])
        bias_t = {}
        for lvl in range(1, NLVL + 1):
            bcols = LVL_NODES[lvl - 1] if lvl < NLVL else LVL_NODES[lvl - 1] // 2
            bias_t[lvl] = const.tile([128, bcols], F32, tag=f"bias{lvl}", name=f"bias{lvl}")
            nc.sync.dma_start(bias_t[lvl][:], p[f"b{lvl}"][:])

        # ---------------- input staging: 4 chunk DMAs ----------------
        a0s = inpool.tile([128, 8192], BF16, tag="a0w5", name="a0s")
        for h in range(4):
            nc.sync.dma_start(a0s[:, h * 2048 : (h + 1) * 2048],
                              p["a0"][:, h * 2048 : (h + 1) * 2048])
        a0v = a0s[:].rearrange("p (h x t) -> p h x t", h=4, x=64)

        # w1 (needed almost immediately by level 1)
        w1t = wpool.tile([128, 1024], BF16, tag="wch", name="w1")
        nc.sync.dma_start(w1t[:, :512], p["w1"][:])

        # ---------------- input conv + level 1, interleaved per half -----
        # X slab: [128=(y%2,c), (b, x=64, t=32)]
        X = apool.tile([128, BG * 64 * 32], BF16, tag="s0", name="x0")
        Xv = X[:].rearrange("p (b h w) -> p b h w", b=BG, h=64)
        A1 = apool.tile([128, 4 * BG * 32 * 16], BF16, tag="s1", name="a1")
        A1v = A1[:].rearrange("p (n b h w) -> p n b h w", n=4, b=BG, h=32)

        for half in range(4):
            # input conv for b = half*4 + g, g = 0..3 concurrent row groups
            for g in range(4):
                b = half * 4 + g
                for xh in range(4):
                    pt = psA.tile([128, 512], F32, tag="psA",
                                  padded_shape=[128, 512],
                                  name=f"pin{b}_{xh}")
                    rhs = a0v[g * 32 : (g + 1) * 32, half,
                              xh * 16 : (xh + 1) * 16, :]
                    nc.tensor.matmul(
                        pt[:], fin_t[g * 32 : (g + 1) * 32, :], rhs,
                        start=True, stop=True,
                        tile_position=(g * 32, 0),
                    )
                    evict(Xv[:, b, xh * 16 : (xh + 1) * 16, :], pt[:],
                          bin_t[:, 0:1])
            # level 1 for the two b-pairs of this half (all 4 nodes)
            for n in range(4):
                for bp in range(2):
                    b0 = half * 4 + bp * 2
                    pt = psB.tile([128, 1024], F32, tag="psB",
                                  padded_shape=[128, 1024],
                                  name=f"p1_{n}_{b0}")
                    for i in range(2):
                        bb = b0 + i
                        for x in (0, 1):
                            for q in (0, 1):
                                rhs = Xv[:, bb, x::2, q::2]
                                nc.tensor.matmul(
                                    pt[q * 64 : (q + 1) * 64,
                                       i * 512 : (i + 1) * 512],
                                    w1t[:, n * 128 + x * 64 :
                                        n * 128 + (x + 1) * 64],
                                    rhs,
                                    start=(x == 0), stop=(x == 1),
                                    skip_group_check=True,
                                    tile_position=(0, q * 64),
                                )
                    evict(A1v[:, n, b0 : b0 + 2, :, :], pt[:],
                          bias_t[1][:, n : n + 1])

        # ---------------- weight DMAs for later levels ----------------
        w2c = []
        for h in range(2):
            w2t = wpool.tile([128, 1024], BF16, tag="wch", name=f"w2_{h}")
            nc.sync.dma_start(w2t[:], p["w2"][:, h * 1024 : (h + 1) * 1024])
            w2c.append(w2t)
        w4t = pfpool.tile([128, 8192], BF16, tag="pf", name="w4")
        for h in range(2):
            nc.sync.dma_start(w4t[:, h * 4096 : (h + 1) * 4096],
                              p["w4"][:, h * 4096 : (h + 1) * 4096])
        w6t = pfpool.tile([128, 8192], BF16, tag="pf", name="w6")
        for h in range(2):
            nc.sync.dma_start(w6t[:, h * 4096 : (h + 1) * 4096],
                              p["w6"][:, h * 4096 : (h + 1) * 4096])
        wdt = pdpool.tile([128, 4096], BF16, tag="wd", name="wd")
        nc.sync.dma_start(wdt[:], p["wd"][:])
        # w5 reuses the input-blob slot (waits for input conv to finish)
        w5t = inpool.tile([128, 8192], BF16, tag="a0w5", name="w5")
        for h in range(2):
            nc.sync.dma_start(w5t[:, h * 4096 : (h + 1) * 4096],
                              p["w5"][:, h * 4096 : (h + 1) * 4096])

        # ---------------- level 2 ----------------
        A2 = apool.tile([128, 16 * BG * 16 * 8], BF16, tag="s0", name="a2")
        A2v = A2[:].rearrange("p (n b h w) -> p n b h w", n=16, b=BG, h=16)
        for n in range(16):
            pn = (n // 4 // 2) * 2 + (n % 4) // 2
            for t in range(2):
                pt = psB.tile([128, 1024], F32, tag="psB",
                              padded_shape=[128, 1024], name=f"p2_{n}_{t}")
                for i in range(2):
                    bs = t * 8 + i * 4
                    for x in (0, 1):
                        for q in (0, 1):
                            rhs = A1v[:, pn, bs : bs + 4, x::2, q::2]
                            nl = n % 8
                            nc.tensor.matmul(
                                pt[q * 64 : (q + 1) * 64,
                                   i * 512 : (i + 1) * 512],
                                w2c[n // 8][:, nl * 128 + x * 64 :
                                            nl * 128 + (x + 1) * 64],
                                rhs,
                                start=(x == 0), stop=(x == 1),
                                skip_group_check=True,
                                tile_position=(0, q * 64),
                            )
                evict(A2v[:, n, t * 8 : (t + 1) * 8, :, :], pt[:],
                      bias_t[2][:, n : n + 1])

        # ---------------- level 3 (w3 streamed in chunks) ----------------
        A3 = apool.tile([128, 64 * BG * 8 * 4], BF16, tag="s1", name="a3")
        A3v = A3[:].rearrange("p (n b h w) -> p n b h w", n=64, b=BG, h=8)
        for g0 in range(0, 64, WGRP):
            w3t = wpool.tile([128, 1024], BF16, tag="wch", name=f"w3_{g0}")
            nc.sync.dma_start(w3t[:], p["w3"][:, g0 * 128 : (g0 + WGRP) * 128])
            for n in range(g0, g0 + WGRP):
                ln = n - g0
                pn = (n // 8 // 2) * 4 + (n % 8) // 2
                pt = psA.tile([128, 512], F32, tag="psA",
                              padded_shape=[128, 512], name=f"p3_{n}")
                for x in (0, 1):
                    for q in (0, 1):
                        rhs = A2v[:, pn, :, x::2, q::2]
                        nc.tensor.matmul(
                            pt[q * 64 : (q + 1) * 64, :],
                            w3t[:, ln * 128 + x * 64 : ln * 128 + (x + 1) * 64],
                            rhs,
                            start=(x == 0), stop=(x == 1),
                            skip_group_check=True,
                            tile_position=(0, q * 64),
                        )
                evict(A3v[:, n, :, :, :], pt[:], bias_t[3][:, n : n + 1])

        # ---------------- level 4 ----------------
        A4 = apool.tile([128, 64 * BG * 4 * 2], BF16, tag="s0", name="a4")
        A4v = A4[:].rearrange("p (n b h w) -> p n b h w", n=64, b=BG, h=4)
        if zero_deep_bias:
            for grp in range(0, 64, 4):
                pt = psA.tile([128, 512], F32, tag="psA",
                              padded_shape=[128, 512], name=f"p4_{grp}")
                for j in range(4):
                    n = grp + j
                    for x in (0, 1):
                        for q in (0, 1):
                            rhs = A3v[:, n, :, x::2, q::2]
                            nc.tensor.matmul(
                                pt[q * 64 : (q + 1) * 64,
                                   j * 128 : (j + 1) * 128],
                                w4t[:, n * 128 + x * 64 :
                                    n * 128 + (x + 1) * 64],
                                rhs,
                                start=(x == 0), stop=(x == 1),
                                skip_group_check=True,
                                tile_position=(0, q * 64),
                            )
                evict(A4v[:, grp : grp + 4, :, :, :], pt[:], None)
        else:
            for n in range(64):
                pt = psA.tile([128, 512], F32, tag="psA",
                              padded_shape=[128, 512], name=f"p4_{n}")
                for x in (0, 1):
                    for q in (0, 1):
                        rhs = A3v[:, n, :, x::2, q::2]
                        nc.tensor.matmul(
                            pt[q * 64 : (q + 1) * 64, :128],
                            w4t[:, n * 128 + x * 64 : n * 128 + (x + 1) * 64],
                            rhs,
                            start=(x == 0), stop=(x == 1),
                            skip_group_check=True,
                            tile_position=(0, q * 64),
                        )
                evict(A4v[:, n, :, :, :], pt[:, :128], bias_t[4][:, n : n + 1])

        # ---------------- level 5 ----------------
        A5 = apool.tile([128, 64 * BG * 2 * 1], BF16, tag="s1", name="a5")
        A5v = A5[:].rearrange("p (n b h w) -> p n b h w", n=64, b=BG, h=2)
        if zero_deep_bias:
            for grp in range(0, 64, 16):
                pt = psA.tile([128, 512], F32, tag="psA",
                              padded_shape=[128, 512], name=f"p5_{grp}")
                for j in range(16):
                    n = grp + j
                    for x in (0, 1):
                        for q in (0, 1):
                            rhs = A4v[:, n, :, x::2, q::2]
                            nc.tensor.matmul(
                                pt[q * 64 : (q + 1) * 64,
                                   j * 32 : (j + 1) * 32],
                                w5t[:, n * 128 + x * 64 :
                                    n * 128 + (x + 1) * 64],
                                rhs,
                                start=(x == 0), stop=(x == 1),
                                skip_group_check=True,
                                tile_position=(0, q * 64),
                            )
                evict(A5v[:, grp : grp + 16, :, :, :], pt[:], None)
        else:
            for n in range(64):
                pt = psA.tile([128, 512], F32, tag="psA",
                              padded_shape=[128, 512], name=f"p5_{n}")
                for x in (0, 1):
                    for q in (0, 1):
                        rhs = A4v[:, n, :, x::2, q::2]
                        nc.tensor.matmul(
                            pt[q * 64 : (q + 1) * 64, :32],
                            w5t[:, n * 128 + x * 64 : n * 128 + (x + 1) * 64],
                            rhs,
                            start=(x == 0), stop=(x == 1),
                            skip_group_check=True,
                            tile_position=(0, q * 64),
                        )
                evict(A5v[:, n, :, :, :], pt[:, :32], bias_t[5][:, n : n + 1])

        # ---------------- level 6: feats F2 [128=(s,c), (pair, b)] -------
        F2 = fpool.tile([128, 32 * BG], BF16, tag="feats", name="f2")
        F2v = F2[:].rearrange("p (n b) -> p n b", n=32)
        if zero_deep_bias:
            for p0 in range(0, 32, 4):
                pt = psA.tile([128, 4 * BG], F32, tag="psA",
                              padded_shape=[128, 512], name=f"p6_{p0}")
                for j in range(4):
                    pr = p0 + j
                    for x in (0, 1):
                        for s in (0, 1):
                            node = 2 * pr + s
                            rhs = A5v[:, node, :, x, 0]
                            nc.tensor.matmul(
                                pt[s * 64 : (s + 1) * 64,
                                   j * BG : (j + 1) * BG],
                                w6t[:, node * 128 + x * 64 :
                                    node * 128 + (x + 1) * 64],
                                rhs,
                                start=(x == 0), stop=(x == 1),
                                skip_group_check=True,
                                tile_position=(0, s * 64),
                            )
                evict(F2v[:, p0 : p0 + 4, :], pt[:], None)
        else:
            for pr in range(32):
                pt = psA.tile([128, BG], F32, tag="psA",
                              padded_shape=[128, 512], name=f"p6_{pr}")
                for x in (0, 1):
                    for s in (0, 1):
                        node = 2 * pr + s
                        rhs = A5v[:, node, :, x, 0]
                        nc.tensor.matmul(
                            pt[s * 64 : (s + 1) * 64, :],
                            w6t[:, node * 128 + x * 64 :
                                node * 128 + (x + 1) * 64],
                            rhs,
                            start=(x == 0), stop=(x == 1),
                            skip_group_check=True,
                            tile_position=(0, s * 64),
                        )
                evict(F2v[:, pr, :], pt[:, :BG], bias_t[6][:, pr : pr + 1])

        # ---------------- dense (row-tiled even/odd nodes) ----------------
        # t2s [128=(r,ou,ov), (s, pair, b)]
        t2s = {}
        for s in (0, 1):
            t2s[s] = opool.tile([128, 32 * BG], F32, tag="t2s", name=f"t2s{s}")
        for p0 in range(0, 32, 4):
            pts = {}
            for s in (0, 1):
                pts[s] = psA.tile([128, 4 * BG], F32, tag="psA",
                                  padded_shape=[128, 512], name=f"pd{s}_{p0}")
            for j in range(4):
                pr = p0 + j
                for s in (0, 1):
                    nc.tensor.matmul(
                        pts[s][:, j * BG : (j + 1) * BG],
                        wdt[s * 64 : (s + 1) * 64, pr * 128 : (pr + 1) * 128],
                        F2v[s * 64 : (s + 1) * 64, pr, :],
                        start=True, stop=True,
                        tile_position=(s * 64, 0),
                    )
            for s in (0, 1):
                evict_ctr[0] += 1
                dst = t2s[s][:, p0 * BG : (p0 + 4) * BG]
                if evict_ctr[0] % 2 == 0:
                    nc.scalar.copy(dst, pts[s][:])
                else:
                    nc.vector.tensor_copy(dst, pts[s][:])
        for s in (0, 1):
            nc.sync.dma_start(
                t2[:, s * 32 * BG : (s + 1) * 32 * BG], t2s[s][:]
            )
    nc.compile()
    return nc


# ----------------------------------------------------------------------------
# entry point
# ----------------------------------------------------------------------------

def _zero_deep_bias(inputs):
    return all(
        float(np.abs(np.asarray(inputs[k])).max()) == 0.0
        for k in ("b4", "b5", "b6")
    )


def kernel(**inputs):
    inputs = {k: np.asarray(v) for k, v in inputs.items()}
    wblobs = _prep_weights(inputs)
    nc = _build_kernel(zero_deep_bias=_zero_deep_bias(inputs))
    in_maps = []
    for c in range(NCORES):
        m = dict(wblobs)
        m["a0"] = _prep_input(inputs["in_data"][c * BC : (c + 1) * BC])
        in_maps.append(m)
    res = run_bass_kernel_spmd(nc, in_maps, list(range(NCORES)))
    outs = [_decode_output(res.results[c]["t2"]) for c in range(NCORES)]
    return np.concatenate(outs, axis=0).astype(np.float32)


if __name__ == "__main__":
    import reference as ref

    inputs = {k: np.asarray(v) for k, v in ref.setup_inputs().items()}
    expected = np.asarray(ref.reference(**inputs))
    actual = kernel(**inputs)
    err = np.abs(actual - expected).max()
    rel = err / np.abs(expected).max()
    print("absmax:", err, "rel:", rel)


# revision 9
# speedup vs baseline: 1.5211x; 1.1424x over previous
"""Trainium2 Bass kernel for nn_ButterflyLayer2D (butterfly 2D CNN).

Strategy: pure data parallel over 8 NeuronCores (16 batch each).

Layouts (per core):
  - activations in SBUF as [128 = (w%2)*64 + c, (node, b, h, w//2)] so each
    2x2-stride-2 per-node conv is a q-parity scheme: K=128=(y,ci) in
    partitions, x (h-parity) accumulated over 2 matmuls, q (output w-parity)
    split over two col-tiled M=64 matmuls running concurrently on the PE.
    PSUM partitions then equal the destination layout, so every PSUM tile is
    evicted by ONE full-width contiguous relu+bias op (alternating
    ScalarE/VectorE).
  - input 4x4-patch conv: patches paired along y so K=32 (2 patches x 16)
    with block-diagonal weights and M=128=(y%2, c); 4 batch-groups run
    concurrently at row offsets 0/32/64/96. Input DMA blob is a fully-packed
    [128, 8192] slab. Input conv of batch-half h is software-pipelined with
    level 1 of half h-1 so the PE never idles long enough for the HAM clock
    gate to re-throttle.
  - level 6 produces feats F2 [128=(node%2, c), (pair, b)]; the dense layer
    is row-tiled: even/odd nodes of a pair at row offsets 0/64 run
    concurrently with per-node [64,128] weights.
  - deep weights (w4, w6, wd) are prefetched from kernel start on the GpSimd
    DMA queue; w5 reuses the input-blob SBUF slot once the input conv is
    done. All PSUM tiles are [128,512] (one bank) in an 8-deep pool.
"""

import numpy as np
from contextlib import ExitStack

import concourse.bass as bass
import concourse.tile as tile
from concourse import bacc, mybir
from concourse.bass_utils import run_bass_kernel_spmd

F32 = mybir.dt.float32
BF16 = mybir.dt.bfloat16
AF = mybir.ActivationFunctionType
ALU = mybir.AluOpType

B, IN, NLVL, KLVL, C = 128, 256, 6, 3, 64
NK, OU, OV = 8, 8, 8
NCORES = 8
BC = B // NCORES          # 16 per-core batch
BG = BC
LVL_NODES = [4, 16, 64, 64, 64, 64]
LVL_HIN = [64, 32, 16, 8, 4, 2]
WGRP = 8                  # w3 streaming chunk (nodes)
BIAS_COLS = [4, 16, 64, 64, 64, 32]   # cols of each level's bias blob


# ----------------------------------------------------------------------------
# host-side pre-arrangement
# ----------------------------------------------------------------------------

def _prep_weights(inputs):
    """Weights/biases blobs shared by all cores."""
    import ml_dtypes
    out = {}
    # input filter: block-diagonal pair lhsT [32=(s,p,q), 128=(s,c)],
    # replicated at row bases 0/32/64/96 (4 concurrent batch groups)
    fin = inputs["in_filter"][:, :, 0, :].reshape(16, C).astype(np.float32)
    blk = np.zeros((32, 128), np.float32)
    blk[0:16, 0:64] = fin
    blk[16:32, 64:128] = fin
    finr = np.zeros((128, 128), np.float32)
    for g in range(4):
        finr[g * 32 : (g + 1) * 32] = blk
    out["fin"] = finr.astype(ml_dtypes.bfloat16)
    out["bin"] = np.concatenate([inputs["in_bias"], inputs["in_bias"]]).reshape(
        128, 1
    ).astype(np.float32)

    bias_blobs = []
    for lvl in range(1, NLVL + 1):
        f = inputs[f"f{lvl}"].astype(np.float32)  # [n,n,2,2,C,C] (x,y,ci,co)
        n = f.shape[0]
        assert n == 2 ** min(lvl, KLVL)
        # lhsT per node: [(y*64+ci), (x*64+co)] -> blob [128, n2*128]
        w = f.transpose(0, 1, 3, 4, 2, 5).reshape(n * n, 2 * C, 2 * C)
        out[f"w{lvl}"] = np.ascontiguousarray(w.transpose(1, 0, 2)).reshape(
            128, n * n * 128
        ).astype(ml_dtypes.bfloat16)
        b = inputs[f"b{lvl}"].astype(np.float32).reshape(n * n, C)
        if lvl < NLVL:
            bb = np.concatenate([b, b], axis=1)  # [nodes, 128] rows (q,c) dup
            bias_blobs.append(np.ascontiguousarray(bb.T))
        else:
            bb = b.reshape(n * n // 2, 2 * C)    # [pairs, (cA,cB)]
            bias_blobs.append(np.ascontiguousarray(bb.T))
    out["biases"] = np.ascontiguousarray(np.concatenate(bias_blobs, axis=1))
    # dense row-tiled: wd2 [128 = s*64 + c, pair*128 + (r*64 + ou*8 + ov)]
    wd = inputs["Wd"].astype(np.float32).reshape(NK * NK, 2, C, OU * OV)
    wd = wd.reshape(32, 2, 2, C, OU * OV)          # [pair, s, r, c, k2]
    wd = wd.transpose(1, 3, 0, 2, 4)               # [s, c, pair, r, k2]
    out["wd"] = np.ascontiguousarray(wd).reshape(128, 32 * 128).astype(
        ml_dtypes.bfloat16
    )
    return out


def _prep_input(in_data_core):
    """Per-core input blob [128, 8192]:
    row = (b%4)*32 + s*16 + p*4 + q ; col = (b//4)*2048 + x*32 + t
    value = in[b, 4x+p, 8t+4s+q]."""
    import ml_dtypes
    ind = in_data_core[:, :, :, 0]  # [16, 256, 256]
    a = ind.reshape(4, 4, 64, 4, 32, 2, 4)      # [half, g, x, p, t, s, q]
    a = a.transpose(1, 5, 3, 6, 0, 2, 4)        # [g, s, p, q, half, x, t]
    return np.ascontiguousarray(a).reshape(128, 8192).astype(ml_dtypes.bfloat16)


def _decode_output(t2_core):
    """t2 [128=(r,ou,ov), (s, pair, b)] -> [16, 64, 64, 2]."""
    t = t2_core.reshape(2, OU, OV, 2, 32, BG)   # r,ou,ov,s,p,b
    t = t.transpose(4, 3, 5, 0, 1, 2)           # p,s,b,r,ou,ov
    t = np.ascontiguousarray(t).reshape(8, 8, BG, 2, OU, OV)  # u,v,b,r,ou,ov
    t = t.transpose(2, 0, 4, 1, 5, 3)           # b,u,ou,v,ov,r
    return np.ascontiguousarray(t).reshape(BC, NK * OU, NK * OV, 2)


# ----------------------------------------------------------------------------
# device kernel
# ----------------------------------------------------------------------------

def _build_kernel(zero_deep_bias=True):
    nc = bacc.Bacc(None, target_bir_lowering=False)
    p = {}
    p["a0"] = nc.declare_dram_parameter("a0", [128, 8192], BF16, isOutput=False)
    p["fin"] = nc.declare_dram_parameter("fin", [128, 128], BF16, isOutput=False)
    p["bin"] = nc.declare_dram_parameter("bin", [128, 1], F32, isOutput=False)
    for lvl in range(1, NLVL + 1):
        n2 = LVL_NODES[lvl - 1]
        p[f"w{lvl}"] = nc.declare_dram_parameter(f"w{lvl}", [128, n2 * 128], BF16, isOutput=False)
    p["biases"] = nc.declare_dram_parameter("biases", [128, sum(BIAS_COLS)], F32, isOutput=False)
    p["wd"] = nc.declare_dram_parameter("wd", [128, 32 * 128], BF16, isOutput=False)
    t2 = nc.declare_dram_parameter("t2", [128, 2 * 32 * BG], F32, isOutput=True)

    evict_ctr = [0]

    def evict(out_ap, psum_ap, bias_ap):
        """relu(psum + bias) -> sbuf, alternating engines to split the load.
        bias_ap None -> plain relu."""
        evict_ctr[0] += 1
        if evict_ctr[0] % 2 == 0:
            if bias_ap is None:
                nc.scalar.activation(out_ap, psum_ap, AF.Relu)
            else:
                nc.scalar.activation(out_ap, psum_ap, AF.Relu, bias=bias_ap)
        else:
            if bias_ap is None:
                nc.vector.tensor_scalar_max(out_ap, psum_ap, 0.0)
            else:
                nc.vector.tensor_scalar(out_ap, psum_ap, bias_ap, 0.0,
                                        op0=ALU.add, op1=ALU.max)

    with tile.TileContext(nc) as tc, ExitStack() as ctx:
        const = ctx.enter_context(tc.tile_pool(name="const", bufs=1))
        wpool = ctx.enter_context(tc.tile_pool(name="wts", bufs=3))
        apool = ctx.enter_context(tc.tile_pool(name="acts", bufs=1))
        inpool = ctx.enter_context(tc.tile_pool(name="inp", bufs=1))
        pfpool = ctx.enter_context(tc.tile_pool(name="pf", bufs=2))
        pdpool = ctx.enter_context(tc.tile_pool(name="pfd", bufs=1))
        fpool = ctx.enter_context(tc.tile_pool(name="feat", bufs=1))
        opool = ctx.enter_context(tc.tile_pool(name="outp", bufs=2))
        psA = ctx.enter_context(tc.tile_pool(name="psA", bufs=8, space="PSUM"))

        def ptile(name):
            return psA.tile([128, 512], F32, tag="psA",
                            padded_shape=[128, 512], name=name)

        # ---------------- input + constants DMA (critical path order) -----
        a0s = inpool.tile([128, 8192], BF16, tag="a0w5", name="a0s")
        nc.sync.dma_start(a0s[:, 0:2048], p["a0"][:, 0:2048])
        fin_t = const.tile([128, 128], BF16)
        nc.sync.dma_start(fin_t[:], p["fin"][:])
        bin_t = const.tile([128, 1], F32)
        nc.sync.dma_start(bin_t[:], p["bin"][:])
        ball_t = const.tile([128, sum(BIAS_COLS)], F32, tag="biases", name="biases")
        nc.sync.dma_start(ball_t[:], p["biases"][:])
        boff = {}
        off = 0
        for lvl in range(1, NLVL + 1):
            boff[lvl] = off
            off += BIAS_COLS[lvl - 1]

        def bslice(lvl, n):
            return ball_t[:, boff[lvl] + n : boff[lvl] + n + 1]
        for h in range(1, 4):
            nc.sync.dma_start(a0s[:, h * 2048 : (h + 1) * 2048],
                              p["a0"][:, h * 2048 : (h + 1) * 2048])
        a0v = a0s[:].rearrange("p (h x t) -> p h x t", h=4, x=64)
        w1t = wpool.tile([128, 1024], BF16, tag="wch", name="w1")
        nc.sync.dma_start(w1t[:, :512], p["w1"][:])

        # bulk weight prefetch on the gpsimd DMA queue
        w2c = []
        for h in range(2):
            w2t = wpool.tile([128, 1024], BF16, tag="wch", name=f"w2_{h}")
            nc.gpsimd.dma_start(w2t[:], p["w2"][:, h * 1024 : (h + 1) * 1024])
            w2c.append(w2t)
        w4t = pfpool.tile([128, 8192], BF16, tag="pf", name="w4")
        w6t = pfpool.tile([128, 8192], BF16, tag="pf", name="w6")
        for h in range(2):
            nc.gpsimd.dma_start(w4t[:, h * 4096 : (h + 1) * 4096],
                                p["w4"][:, h * 4096 : (h + 1) * 4096])
        for h in range(2):
            nc.gpsimd.dma_start(w6t[:, h * 4096 : (h + 1) * 4096],
                                p["w6"][:, h * 4096 : (h + 1) * 4096])
        wdt = pdpool.tile([128, 4096], BF16, tag="wd", name="wd")
        nc.gpsimd.dma_start(wdt[:], p["wd"][:])
        # w5 reuses the input-blob slot (waits for input conv to finish)
        w5t = inpool.tile([128, 8192], BF16, tag="a0w5", name="w5")
        for h in range(2):
            nc.gpsimd.dma_start(w5t[:, h * 4096 : (h + 1) * 4096],
                                p["w5"][:, h * 4096 : (h + 1) * 4096])

        # ---------------- input conv + level 1, software pipelined --------
        # X slab: [128=(y%2,c), (b, x=64, t=32)]
        X = apool.tile([128, BG * 64 * 32], BF16, tag="s0", name="x0")
        Xv = X[:].rearrange("p (b h w) -> p b h w", b=BG, h=64)
        A1 = apool.tile([128, 4 * BG * 32 * 16], BF16, tag="s1", name="a1")
        A1v = A1[:].rearrange("p (n b h w) -> p n b h w", n=4, b=BG, h=32)

        def input_conv(half):
            for xh in range(4):
                for g in range(4):
                    b = half * 4 + g
                    pt = ptile(f"pin{b}_{xh}")
                    rhs = a0v[g * 32 : (g + 1) * 32, half,
                              xh * 16 : (xh + 1) * 16, :]
                    nc.tensor.matmul(
                        pt[:], fin_t[g * 32 : (g + 1) * 32, :], rhs,
                        start=True, stop=True,
                        tile_position=(g * 32, 0),
                    )
                    evict(Xv[:, b, xh * 16 : (xh + 1) * 16, :], pt[:],
                          bin_t[:, 0:1])

        def lvl1(half):
            for n in range(4):
                for i in range(4):
                    bb = half * 4 + i
                    pt = ptile(f"p1_{n}_{bb}")
                    for x in (0, 1):
                        for q in (0, 1):
                            rhs = Xv[:, bb, x::2, q::2]
                            nc.tensor.matmul(
                                pt[q * 64 : (q + 1) * 64, :],
                                w1t[:, n * 128 + x * 64 :
                                    n * 128 + (x + 1) * 64],
                                rhs,
                                start=(x == 0), stop=(x == 1),
                                skip_group_check=True,
                                tile_position=(0, q * 64),
                            )
                    evict(A1v[:, n, bb, :, :], pt[:], bslice(1, n))

        input_conv(0)
        for half in range(1, 4):
            input_conv(half)
            lvl1(half - 1)
        lvl1(3)

        # ---------------- level 2 ----------------
        A2 = apool.tile([128, 16 * BG * 16 * 8], BF16, tag="s0", name="a2")
        A2v = A2[:].rearrange("p (n b h w) -> p n b h w", n=16, b=BG, h=16)
        for n in range(16):
            pn = (n // 4 // 2) * 2 + (n % 4) // 2
            nl = n % 8
            for t in range(4):
                pt = ptile(f"p2_{n}_{t}")
                for x in (0, 1):
                    for q in (0, 1):
                        rhs = A1v[:, pn, t * 4 : (t + 1) * 4, x::2, q::2]
                        nc.tensor.matmul(
                            pt[q * 64 : (q + 1) * 64, :],
                            w2c[n // 8][:, nl * 128 + x * 64 :
                                        nl * 128 + (x + 1) * 64],
                            rhs,
                            start=(x == 0), stop=(x == 1),
                            skip_group_check=True,
                            tile_position=(0, q * 64),
                        )
                evict(A2v[:, n, t * 4 : (t + 1) * 4, :, :], pt[:],
                      bslice(2, n))

        # ---------------- level 3 (w3 streamed in chunks) ----------------
        A3 = apool.tile([128, 64 * BG * 8 * 4], BF16, tag="s1", name="a3")
        A3v = A3[:].rearrange("p (n b h w) -> p n b h w", n=64, b=BG, h=8)
        for g0 in range(0, 64, WGRP):
            w3t = wpool.tile([128, 1024], BF16, tag="wch", name=f"w3_{g0}")
            nc.sync.dma_start(w3t[:], p["w3"][:, g0 * 128 : (g0 + WGRP) * 128])
            for n in range(g0, g0 + WGRP):
                ln = n - g0
                pn = (n // 8 // 2) * 4 + (n % 8) // 2
                pt = ptile(f"p3_{n}")
                for x in (0, 1):
                    for q in (0, 1):
                        rhs = A2v[:, pn, :, x::2, q::2]
                        nc.tensor.matmul(
                            pt[q * 64 : (q + 1) * 64, :],
                            w3t[:, ln * 128 + x * 64 : ln * 128 + (x + 1) * 64],
                            rhs,
                            start=(x == 0), stop=(x == 1),
                            skip_group_check=True,
                            tile_position=(0, q * 64),
                        )
                evict(A3v[:, n, :, :, :], pt[:], bslice(3, n))

        # ---------------- level 4 ----------------
        A4 = apool.tile([128, 64 * BG * 4 * 2], BF16, tag="s0", name="a4")
        A4v = A4[:].rearrange("p (n b h w) -> p n b h w", n=64, b=BG, h=4)
        if zero_deep_bias:
            for grp in range(0, 64, 4):
                pt = ptile(f"p4_{grp}")
                for j in range(4):
                    n = grp + j
                    for x in (0, 1):
                        for q in (0, 1):
                            rhs = A3v[:, n, :, x::2, q::2]
                            nc.tensor.matmul(
                                pt[q * 64 : (q + 1) * 64,
                                   j * 128 : (j + 1) * 128],
                                w4t[:, n * 128 + x * 64 :
                                    n * 128 + (x + 1) * 64],
                                rhs,
                                start=(x == 0), stop=(x == 1),
                                skip_group_check=True,
                                tile_position=(0, q * 64),
                            )
                evict(A4v[:, grp : grp + 4, :, :, :], pt[:], None)
        else:
            for n in range(64):
                pt = ptile(f"p4_{n}")
                for x in (0, 1):
                    for q in (0, 1):
                        rhs = A3v[:, n, :, x::2, q::2]
                        nc.tensor.matmul(
                            pt[q * 64 : (q + 1) * 64, :128],
                            w4t[:, n * 128 + x * 64 : n * 128 + (x + 1) * 64],
                            rhs,
                            start=(x == 0), stop=(x == 1),
                            skip_group_check=True,
                            tile_position=(0, q * 64),
                        )
                evict(A4v[:, n, :, :, :], pt[:, :128], bslice(4, n))

        # ---------------- level 5 ----------------
        A5 = apool.tile([128, 64 * BG * 2 * 1], BF16, tag="s1", name="a5")
        A5v = A5[:].rearrange("p (n b h w) -> p n b h w", n=64, b=BG, h=2)
        if zero_deep_bias:
            for grp in range(0, 64, 16):
                pt = ptile(f"p5_{grp}")
                for j in range(16):
                    n = grp + j
                    for x in (0, 1):
                        for q in (0, 1):
                            rhs = A4v[:, n, :, x::2, q::2]
                            nc.tensor.matmul(
                                pt[q * 64 : (q + 1) * 64,
                                   j * 32 : (j + 1) * 32],
                                w5t[:, n * 128 + x * 64 :
                                    n * 128 + (x + 1) * 64],
                                rhs,
                                start=(x == 0), stop=(x == 1),
                                skip_group_check=True,
                                tile_position=(0, q * 64),
                            )
                evict(A5v[:, grp : grp + 16, :, :, :], pt[:], None)
        else:
            for n in range(64):
                pt = ptile(f"p5_{n}")
                for x in (0, 1):
                    for q in (0, 1):
                        rhs = A4v[:, n, :, x::2, q::2]
                        nc.tensor.matmul(
                            pt[q * 64 : (q + 1) * 64, :32],
                            w5t[:, n * 128 + x * 64 : n * 128 + (x + 1) * 64],
                            rhs,
                            start=(x == 0), stop=(x == 1),
                            skip_group_check=True,
                            tile_position=(0, q * 64),
                        )
                evict(A5v[:, n, :, :, :], pt[:, :32], bslice(5, n))

        # ---------------- level 6: feats F2 [128=(s,c), (pair, b)] -------
        F2 = fpool.tile([128, 32 * BG], BF16, tag="feats", name="f2")
        F2v = F2[:].rearrange("p (n b) -> p n b", n=32)
        if zero_deep_bias:
            for p0 in range(0, 32, 4):
                pt = ptile(f"p6_{p0}")
                for j in range(4):
                    pr = p0 + j
                    for x in (0, 1):
                        for s in (0, 1):
                            node = 2 * pr + s
                            rhs = A5v[:, node, :, x, 0]
                            nc.tensor.matmul(
                                pt[s * 64 : (s + 1) * 64,
                                   j * BG : (j + 1) * BG],
                                w6t[:, node * 128 + x * 64 :
                                    node * 128 + (x + 1) * 64],
                                rhs,
                                start=(x == 0), stop=(x == 1),
                                skip_group_check=True,
                                tile_position=(0, s * 64),
                            )
                evict(F2v[:, p0 : p0 + 4, :], pt[:, : 4 * BG], None)
        else:
            for pr in range(32):
                pt = ptile(f"p6_{pr}")
                for x in (0, 1):
                    for s in (0, 1):
                        node = 2 * pr + s
                        rhs = A5v[:, node, :, x, 0]
                        nc.tensor.matmul(
                            pt[s * 64 : (s + 1) * 64, :BG],
                            w6t[:, node * 128 + x * 64 :
                                node * 128 + (x + 1) * 64],
                            rhs,
                            start=(x == 0), stop=(x == 1),
                            skip_group_check=True,
                            tile_position=(0, s * 64),
                        )
                evict(F2v[:, pr, :], pt[:, :BG], bslice(6, pr))

        # ---------------- dense (row-tiled even/odd nodes) ----------------
        # t2s [128=(r,ou,ov), (s, pair, b)]
        t2s = {}
        for s in (0, 1):
            t2s[s] = opool.tile([128, 32 * BG], F32, tag="t2s", name=f"t2s{s}")
        for p0 in range(0, 32, 4):
            pts = {}
            for s in (0, 1):
                pts[s] = ptile(f"pd{s}_{p0}")
            for j in range(4):
                pr = p0 + j
                for s in (0, 1):
                    nc.tensor.matmul(
                        pts[s][:, j * BG : (j + 1) * BG],
                        wdt[s * 64 : (s + 1) * 64, pr * 128 : (pr + 1) * 128],
                        F2v[s * 64 : (s + 1) * 64, pr, :],
                        start=True, stop=True,
                        tile_position=(s * 64, 0),
                    )
            for s in (0, 1):
                evict_ctr[0] += 1
                dst = t2s[s][:, p0 * BG : (p0 + 4) * BG]
                if evict_ctr[0] % 2 == 0:
                    nc.scalar.copy(dst, pts[s][:, : 4 * BG])
                else:
                    nc.vector.tensor_copy(dst, pts[s][:, : 4 * BG])
        for s in (0, 1):
            nc.sync.dma_start(
                t2[:, s * 32 * BG : (s + 1) * 32 * BG], t2s[s][:]
            )
    nc.compile()
    return nc


# ----------------------------------------------------------------------------
# entry point
# ----------------------------------------------------------------------------

def _zero_deep_bias(inputs):
    return all(
        float(np.abs(np.asarray(inputs[k])).max()) == 0.0
        for k in ("b4", "b5", "b6")
    )


def kernel(**inputs):
    inputs = {k: np.asarray(v) for k, v in inputs.items()}
    wblobs = _prep_weights(inputs)
    nc = _build_kernel(zero_deep_bias=_zero_deep_bias(inputs))
    in_maps = []
    for c in range(NCORES):
        m = dict(wblobs)
        m["a0"] = _prep_input(inputs["in_data"][c * BC : (c + 1) * BC])
        in_maps.append(m)
    res = run_bass_kernel_spmd(nc, in_maps, list(range(NCORES)))
    outs = [_decode_output(res.results[c]["t2"]) for c in range(NCORES)]
    return np.concatenate(outs, axis=0).astype(np.float32)


if __name__ == "__main__":
    import reference as ref

    inputs = {k: np.asarray(v) for k, v in ref.setup_inputs().items()}
    expected = np.asarray(ref.reference(**inputs))
    actual = kernel(**inputs)
    err = np.abs(actual - expected).max()
    rel = err / np.abs(expected).max()
    print("absmax:", err, "rel:", rel)
